# revision 2
# baseline (speedup 1.0000x reference)
"""Trainium2 Bass kernel for batched bilinear (general) attention.

Reference computation (all fp32):
    psi = einsum("bth,ah->bta", h_enc, W_psi) + b_psi        # [B, T, A]
    phi = einsum("qbh,ah->qba", h_dec, W_phi) + b_phi        # [Q, B, A]
    e   = einsum("bta,qba->btq", psi, phi)                   # [B, T, Q]
    a   = softmax(e, axis=1)                                 # over T
    c   = einsum("bth,btq->bqh", h_enc, a)                   # [B, Q, H]

Key algebraic refactor: e[b,t,q] = enc_t . M . dec_q + enc_t . u + (per-q const)
with M = W_psi^T @ W_phi [H,H], u = W_psi^T @ b_phi.  Per-q-column constants
are invariant under softmax over t, so they are dropped.  The host folds the
weights into Z[b] = M @ dec_b^T + u [H, Q] (tiny), and the device only
computes e = enc @ Z, the softmax, and c = enc^T @ softmax(e).

Sharding: data-parallel over batch B=16 across 8 cores (2 batches per core),
no collectives.
"""

import functools
import os
import sys

import numpy as np

for _p in ("/opt/trn_rl_repo", "/root/.axon_site/_ro/trn_rl_repo"):
    if os.path.isdir(_p) and _p not in sys.path:
        sys.path.append(_p)

B, T, Q, H = 16, 2048, 64, 1024
NCORES = 8
BL = B // NCORES  # batches per core
KT = H // 128  # 8 H-tiles (contraction tiles for e)
NT = T // 128  # 16 T-tiles
NC_CHUNK = T // 512  # 4 chunks of 512 along T for e PSUM banks


@functools.lru_cache(maxsize=1)
def _build(loop_n: int = 1):
    import concourse.mybir as mybir
    import concourse.tile as tile
    from concourse import bacc
    from concourse.bass import ts
    from concourse.masks import make_identity

    f32 = mybir.dt.float32
    f16 = mybir.dt.float16

    nc = bacc.Bacc(
        "TRN2",
        target_bir_lowering=False,
        debug=False,
        enable_asserts=False,
        num_devices=NCORES,
    )

    encT_d = nc.dram_tensor("encT", [BL, H, T], f32, kind="ExternalInput")
    encN_d = nc.dram_tensor("encN", [BL, T, H], f16, kind="ExternalInput")
    z_d = nc.dram_tensor("z", [BL, H, Q], f32, kind="ExternalInput")
    c_d = nc.dram_tensor("c", [BL, Q, H], f32, kind="ExternalOutput")

    with tile.TileContext(nc) as tc:
        with (
            tc.tile_pool(name="encT", bufs=2) as p_encT,
            tc.tile_pool(name="encN", bufs=4) as p_encN,
            tc.tile_pool(name="z", bufs=2) as p_z,
            tc.tile_pool(name="eT", bufs=1) as p_eT,
            tc.tile_pool(name="pT", bufs=1) as p_pT,
            tc.tile_pool(name="pN", bufs=2) as p_pN,
            tc.tile_pool(name="outs", bufs=2) as p_out,
            tc.tile_pool(name="stats", bufs=8) as p_stats,
            tc.tile_pool(name="singles", bufs=1) as p_singles,
            tc.tile_pool(name="ps_e", bufs=2, space="PSUM") as ps_e,
            tc.tile_pool(name="ps_tr", bufs=2, space="PSUM") as ps_tr,
            tc.tile_pool(name="ps_c", bufs=2, space="PSUM") as ps_c,
        ):
            ident = p_singles.tile([64, 64], f32)
            make_identity(nc, ident)

            for _ in range(loop_n):
                # ---- phase E: e^T[b] = Z[b]^T @ encT[b]  (contract H) ----
                eTs, rs, encNs, zts = [], [], [], []
                for b in range(BL):
                    z_t = p_z.tile([128, KT, Q], f32, tag="z")
                    nc.sync.dma_start(
                        out=z_t[:],
                        in_=z_d.ap()[b].rearrange("(k p) q -> p k q", p=128),
                    )
                    zts.append(z_t)

                    encT_t = p_encT.tile([128, KT, T], f32, tag="encT")
                    for k in range(KT):
                        nc.sync.dma_start(
                            out=encT_t[:, k, :],
                            in_=encT_d.ap()[b, k * 128 : (k + 1) * 128, :],
                        )

                    eT = p_eT.tile([64, T], f32, tag=f"eT{b}")
                    eTs.append(eT)
                    for c_i in range(NC_CHUNK):
                        e_ps = ps_e.tile([64, 512], f32, tag="e_ps")
                        for k in range(KT):
                            nc.tensor.matmul(
                                e_ps[:],
                                lhsT=z_t[:, k, :],
                                rhs=encT_t[:, k, ts(c_i, 512)],
                                start=(k == 0),
                                stop=(k == KT - 1),
                            )
                        if c_i % 2 == 0:
                            nc.vector.tensor_copy(out=eT[:, ts(c_i, 512)], in_=e_ps[:])
                        else:
                            nc.scalar.copy(out=eT[:, ts(c_i, 512)], in_=e_ps[:])

                    # ---- phase S: softmax stats over T (free dim) ----
                    negm = p_stats.tile([64, 1], f32, tag="negm")
                    nc.vector.reduce_max(
                        out=negm[:], in_=eT[:], axis=mybir.AxisListType.X, negate=True
                    )
                    pT = p_pT.tile([64, T], f32, tag=f"pT{b}")
                    s = p_stats.tile([64, 1], f32, tag="s")
                    nc.scalar.activation(
                        out=pT[:],
                        in_=eT[:],
                        func=mybir.ActivationFunctionType.Exp,
                        bias=negm[:],
                        scale=1.0,
                        accum_out=s[:],
                    )
                    r = p_stats.tile([64, 1], f32, tag="r")
                    nc.vector.reciprocal(out=r[:], in_=s[:])
                    rs.append((pT, r))

                # ---- phases T + C per batch ----
                for b in range(BL):
                    pT, r = rs[b]
                    # transpose p^T [64, T] -> p natural tiles [128, 64] (fp16)
                    pN = p_pN.tile([128, NT, Q], f16, tag="pN")
                    for tt in range(NT):
                        tr_ps = ps_tr.tile([128, 64], f32, tag="tr_ps")
                        nc.tensor.transpose(
                            out=tr_ps[:], in_=pT[:, ts(tt, 128)], identity=ident[:]
                        )
                        nc.vector.tensor_copy(out=pN[:, tt, :], in_=tr_ps[:])

                    # c[b] = p^T @ encN[b]  (contract T), scaled by r on evac
                    c_ps0 = ps_c.tile([64, 512], f32, tag="c_ps0")
                    c_ps1 = ps_c.tile([64, 512], f32, tag="c_ps1")
                    for tt in range(NT):
                        encN_t = p_encN.tile([128, H], f16, tag="encN")
                        nc.sync.dma_start(
                            out=encN_t[:],
                            in_=encN_d.ap()[b, tt * 128 : (tt + 1) * 128, :],
                        )
                        nc.tensor.matmul(
                            c_ps0[:],
                            lhsT=pN[:, tt, :],
                            rhs=encN_t[:, 0:512],
                            start=(tt == 0),
                            stop=(tt == NT - 1),
                        )
                        nc.tensor.matmul(
                            c_ps1[:],
                            lhsT=pN[:, tt, :],
                            rhs=encN_t[:, 512:1024],
                            start=(tt == 0),
                            stop=(tt == NT - 1),
                        )

                    out_t = p_out.tile([64, H], f32, tag="out")
                    nc.vector.tensor_scalar_mul(out_t[:, 0:512], c_ps0[:], r[:])
                    nc.vector.tensor_scalar_mul(out_t[:, 512:1024], c_ps1[:], r[:])
                    nc.sync.dma_start(out=c_d.ap()[b], in_=out_t[:])

    nc.compile()
    return nc


def _host_prep(h_enc, h_dec, W_psi, b_psi, W_phi, b_phi):
    h_enc = np.asarray(h_enc, dtype=np.float32)
    h_dec = np.asarray(h_dec, dtype=np.float32)
    W_psi = np.asarray(W_psi, dtype=np.float64)
    W_phi = np.asarray(W_phi, dtype=np.float64)
    b_phi = np.asarray(b_phi, dtype=np.float64)

    # M = W_psi^T @ W_phi [H, H];  u = W_psi^T @ b_phi [H]
    M = W_psi.T @ W_phi
    u = W_psi.T @ b_phi
    # Z[b, h, q] = sum_k M[h, k] * h_dec[q, b, k] + u[h]
    dec_r = h_dec.astype(np.float64).transpose(2, 1, 0).reshape(H, B * Q)  # [k, b*q]
    Z = (M @ dec_r).reshape(H, B, Q).transpose(1, 0, 2) + u[None, :, None]
    Z = np.ascontiguousarray(Z, dtype=np.float32)  # [B, H, Q]

    encT = np.ascontiguousarray(h_enc.transpose(0, 2, 1))  # [B, H, T] fp32
    encN = h_enc.astype(np.float16)  # [B, T, H] fp16
    return encT, encN, Z


def _in_maps(encT, encN, Z):
    maps = []
    for i in range(NCORES):
        s = slice(i * BL, (i + 1) * BL)
        maps.append({"encT": encT[s], "encN": encN[s], "z": Z[s]})
    return maps


def kernel(h_enc, h_dec, W_psi, b_psi, W_phi, b_phi):
    from concourse.bass_utils import run_bass_kernel_spmd

    encT, encN, Z = _host_prep(h_enc, h_dec, W_psi, b_psi, W_phi, b_phi)
    nc = _build()
    res = run_bass_kernel_spmd(nc, _in_maps(encT, encN, Z), core_ids=list(range(NCORES)))
    out = np.concatenate([res.results[i]["c"] for i in range(NCORES)], axis=0)
    return np.ascontiguousarray(out, dtype=np.float32)


# revision 4
# speedup vs baseline: 186.0443x; 186.0443x over previous
"""Trainium2 Bass kernel for batched bilinear (general) attention.

Reference computation (all fp32):
    psi = einsum("bth,ah->bta", h_enc, W_psi) + b_psi        # [B, T, A]
    phi = einsum("qbh,ah->qba", h_dec, W_phi) + b_phi        # [Q, B, A]
    e   = einsum("bta,qba->btq", psi, phi)                   # [B, T, Q]
    a   = softmax(e, axis=1)                                 # over T
    c   = einsum("bth,btq->bqh", h_enc, a)                   # [B, Q, H]

Key algebraic refactor: e[b,t,q] = enc_t . M . dec_q + enc_t . u + (per-q const)
with M = W_psi^T @ W_phi [H,H], u = W_psi^T @ b_phi.  Per-q-column constants
are invariant under softmax over t, so they are dropped.  The host folds the
weights into Z[b] = M @ dec_b^T + u [H, Q] (tiny), and the device only
computes e = enc @ Z, the softmax, and c = enc^T @ softmax(e).

Sharding: data-parallel over batch B=16 across 8 cores (2 batches per core),
no collectives.
"""

import functools
import os
import sys

import numpy as np

for _p in ("/opt/trn_rl_repo", "/root/.axon_site/_ro/trn_rl_repo"):
    if os.path.isdir(_p) and _p not in sys.path:
        sys.path.append(_p)

B, T, Q, H = 16, 2048, 64, 1024
NCORES = 8
BL = B // NCORES  # batches per core
KT = H // 128  # 8 H-tiles (contraction tiles for e)
NT = T // 128  # 16 T-tiles
NC_CHUNK = T // 512  # 4 chunks of 512 along T for e PSUM banks


@functools.lru_cache(maxsize=2)
def _build(loop_n: int = 1):
    import concourse.mybir as mybir
    import concourse.tile as tile
    from concourse import bacc
    from concourse.bass import ts
    from concourse.masks import make_identity

    f32 = mybir.dt.float32
    f16 = mybir.dt.float16

    nc = bacc.Bacc(
        "TRN2",
        target_bir_lowering=False,
        debug=False,
        enable_asserts=False,
        num_devices=NCORES,
    )

    encT_d = nc.dram_tensor("encT", [BL, H, T], f32, kind="ExternalInput")
    encN_d = nc.dram_tensor("encN", [BL, T, H], f16, kind="ExternalInput")
    z_d = nc.dram_tensor("z", [BL, H, Q], f32, kind="ExternalInput")
    c_d = nc.dram_tensor("c", [BL, Q, H], f32, kind="ExternalOutput")

    with tile.TileContext(nc) as tc:
        with (
            tc.tile_pool(name="encT", bufs=2) as p_encT,
            tc.tile_pool(name="encN", bufs=4) as p_encN,
            tc.tile_pool(name="z", bufs=2) as p_z,
            tc.tile_pool(name="eT", bufs=1) as p_eT,
            tc.tile_pool(name="pT", bufs=1) as p_pT,
            tc.tile_pool(name="pN", bufs=2) as p_pN,
            tc.tile_pool(name="outs", bufs=2) as p_out,
            tc.tile_pool(name="stats", bufs=8) as p_stats,
            tc.tile_pool(name="singles", bufs=1) as p_singles,
            tc.tile_pool(name="ps_e", bufs=2, space="PSUM") as ps_e,
            tc.tile_pool(name="ps_tr", bufs=2, space="PSUM") as ps_tr,
            tc.tile_pool(name="ps_c", bufs=2, space="PSUM") as ps_c,
        ):
            ident = p_singles.tile([64, 64], f32)
            make_identity(nc, ident)

            import contextlib

            loop_ctx = (
                tc.For_i(0, loop_n, 1) if loop_n > 1 else contextlib.nullcontext()
            )
            with loop_ctx:
                # ---- phase E: e^T[b] = Z[b]^T @ encT[b]  (contract H) ----
                eTs, rs, encNs, zts = [], [], [], []
                for b in range(BL):
                    z_t = p_z.tile([128, KT, Q], f32, tag="z")
                    nc.sync.dma_start(
                        out=z_t[:],
                        in_=z_d.ap()[b].rearrange("(k p) q -> p k q", p=128),
                    )
                    zts.append(z_t)

                    encT_t = p_encT.tile([128, KT, T], f32, tag="encT")
                    for k in range(KT):
                        nc.sync.dma_start(
                            out=encT_t[:, k, :],
                            in_=encT_d.ap()[b, k * 128 : (k + 1) * 128, :],
                        )

                    eT = p_eT.tile([64, T], f32, tag=f"eT{b}")
                    eTs.append(eT)
                    for c_i in range(NC_CHUNK):
                        e_ps = ps_e.tile([64, 512], f32, tag="e_ps")
                        for k in range(KT):
                            nc.tensor.matmul(
                                e_ps[:],
                                lhsT=z_t[:, k, :],
                                rhs=encT_t[:, k, ts(c_i, 512)],
                                start=(k == 0),
                                stop=(k == KT - 1),
                            )
                        if c_i % 2 == 0:
                            nc.vector.tensor_copy(out=eT[:, ts(c_i, 512)], in_=e_ps[:])
                        else:
                            nc.scalar.copy(out=eT[:, ts(c_i, 512)], in_=e_ps[:])

                    # ---- phase S: softmax stats over T (free dim) ----
                    negm = p_stats.tile([64, 1], f32, tag="negm")
                    nc.vector.reduce_max(
                        out=negm[:], in_=eT[:], axis=mybir.AxisListType.X, negate=True
                    )
                    pT = p_pT.tile([64, T], f32, tag=f"pT{b}")
                    s = p_stats.tile([64, 1], f32, tag="s")
                    nc.scalar.activation(
                        out=pT[:],
                        in_=eT[:],
                        func=mybir.ActivationFunctionType.Exp,
                        bias=negm[:],
                        scale=1.0,
                        accum_out=s[:],
                    )
                    r = p_stats.tile([64, 1], f32, tag="r")
                    nc.vector.reciprocal(out=r[:], in_=s[:])
                    rs.append((pT, r))

                # ---- phases T + C per batch ----
                for b in range(BL):
                    pT, r = rs[b]
                    # transpose p^T [64, T] -> p natural tiles [128, 64] (fp16)
                    pN = p_pN.tile([128, NT, Q], f16, tag="pN")
                    for tt in range(NT):
                        tr_ps = ps_tr.tile([128, 64], f32, tag="tr_ps")
                        nc.tensor.transpose(
                            out=tr_ps[:], in_=pT[:, ts(tt, 128)], identity=ident[:]
                        )
                        nc.vector.tensor_copy(out=pN[:, tt, :], in_=tr_ps[:])

                    # c[b] = p^T @ encN[b]  (contract T), scaled by r on evac
                    c_ps0 = ps_c.tile([64, 512], f32, tag="c_ps0")
                    c_ps1 = ps_c.tile([64, 512], f32, tag="c_ps1")
                    for tt in range(NT):
                        encN_t = p_encN.tile([128, H], f16, tag="encN")
                        nc.sync.dma_start(
                            out=encN_t[:],
                            in_=encN_d.ap()[b, tt * 128 : (tt + 1) * 128, :],
                        )
                        nc.tensor.matmul(
                            c_ps0[:],
                            lhsT=pN[:, tt, :],
                            rhs=encN_t[:, 0:512],
                            start=(tt == 0),
                            stop=(tt == NT - 1),
                        )
                        nc.tensor.matmul(
                            c_ps1[:],
                            lhsT=pN[:, tt, :],
                            rhs=encN_t[:, 512:1024],
                            start=(tt == 0),
                            stop=(tt == NT - 1),
                        )

                    out_t = p_out.tile([64, H], f32, tag="out")
                    nc.vector.tensor_scalar_mul(out_t[:, 0:512], c_ps0[:], r[:])
                    nc.vector.tensor_scalar_mul(out_t[:, 512:1024], c_ps1[:], r[:])
                    nc.sync.dma_start(out=c_d.ap()[b], in_=out_t[:])

    nc.compile()
    return nc


def _host_prep(h_enc, h_dec, W_psi, b_psi, W_phi, b_phi):
    h_enc = np.asarray(h_enc, dtype=np.float32)
    h_dec = np.asarray(h_dec, dtype=np.float32)
    W_psi = np.asarray(W_psi, dtype=np.float64)
    W_phi = np.asarray(W_phi, dtype=np.float64)
    b_phi = np.asarray(b_phi, dtype=np.float64)

    # M = W_psi^T @ W_phi [H, H];  u = W_psi^T @ b_phi [H]
    M = W_psi.T @ W_phi
    u = W_psi.T @ b_phi
    # Z[b, h, q] = sum_k M[h, k] * h_dec[q, b, k] + u[h]
    dec_r = h_dec.astype(np.float64).transpose(2, 1, 0).reshape(H, B * Q)  # [k, b*q]
    Z = (M @ dec_r).reshape(H, B, Q).transpose(1, 0, 2) + u[None, :, None]
    Z = np.ascontiguousarray(Z, dtype=np.float32)  # [B, H, Q]

    encT = np.ascontiguousarray(h_enc.transpose(0, 2, 1))  # [B, H, T] fp32
    encN = h_enc.astype(np.float16)  # [B, T, H] fp16
    return encT, encN, Z


def _in_maps(encT, encN, Z):
    maps = []
    for i in range(NCORES):
        s = slice(i * BL, (i + 1) * BL)
        maps.append({"encT": encT[s], "encN": encN[s], "z": Z[s]})
    return maps


def kernel(h_enc, h_dec, W_psi, b_psi, W_phi, b_phi):
    from concourse.bass_utils import run_bass_kernel_spmd

    encT, encN, Z = _host_prep(h_enc, h_dec, W_psi, b_psi, W_phi, b_phi)
    nc = _build()
    res = run_bass_kernel_spmd(nc, _in_maps(encT, encN, Z), core_ids=list(range(NCORES)))
    out = np.concatenate([res.results[i]["c"] for i in range(NCORES)], axis=0)
    return np.ascontiguousarray(out, dtype=np.float32)
